# revision 1
# baseline (speedup 1.0000x reference)
"""Trainium2 Bass kernel for nn_Neighbor_Mean (gnn message passing).

Math: out[b,s,:] = mean_n( mask[b,s,n] * (T_b[idx[b,s,n]] @ Wn^T) )
 with T_b[v] = pos_table[v] + (h[b][v-1] if v>=1 else 0)   (v in [0, 2049))
Since the mask multiplies matmul outputs and matmul is linear:
 out[b,s,:] = ( (1/N) * sum_n T'_b[idx_eff[b,s,n]] ) @ Wn^T
 where T' has an extra zero row at SINK=2049 and idx_eff = mask ? idx : SINK.

Sharding: data-parallel over batch, one NeuronCore per batch row (B == 8).

Per-core plan:
 - build T' in SBUF as bf16, packed [128 part, 17*128] (row v at partition
   v%128, free chunk v//128) -- the SBUF-source layout of dma_gather
   (tokens_per_rank=128, free_dim_per_rank=256B).
 - fold mask into indices on DVE (select against SINK), emit int16 in the
   16-partition wrapped layout dma_gather wants, replicate to 128 partitions.
 - SBUF->SBUF transposed dma_gather, 512 idx/call (ucode ring ceiling),
   4 SWDGE queues. Gathered tile g[h=128 part, j free], stream
   j = (n, s%16) per call, call c covers s in [16c, 16c+16).
 - PE: per 128-s chunk, PSUM-accumulate 32 matmuls over n:
   psum[s,k] += g_slice[h, s]^T @ (Wn^T * 1/N) (bf16 x bf16 -> f32).
 - copy PSUM->SBUF, DMA out rows (f32).
"""
import sys

sys.path.insert(0, '/opt/trn_rl_repo')

import numpy as np

import concourse.bacc as bacc
import concourse.bass as bass
import concourse.mybir as mybir
import concourse.tile as tile
from concourse.bass_utils import run_bass_kernel_spmd
from concourse.masks import make_identity

B, N, H = 8, 32, 128
NI = 512             # idxs per dma_gather call (ucode ring ceiling)
SBLK = 512           # s rows per pipeline block
TPR = 128            # sbuf gather tokens per rank
F32 = mybir.dt.float32
I32 = mybir.dt.int32
I16 = mybir.dt.int16
BF16 = mybir.dt.bfloat16


def build_program(S: int = 2048):
    VPOS = S + 1                      # pos_table rows; SINK index == VPOS
    NRANKS = (VPOS + 1 + 127) // 128  # table chunks incl. sink row, padded
    VPAD = NRANKS * 128
    nblk = S // SBLK if S >= SBLK else 1
    sblk = min(SBLK, S)
    calls = sblk * N // NI            # gather calls per block
    chunks = sblk // 128              # 128-s output chunks per block

    nc = bacc.Bacc("TRN2", debug=False, num_swdge_queues=4)
    h_d = nc.dram_tensor("h", [S, H], F32, kind="ExternalInput")
    idx_d = nc.dram_tensor("idx", [S, N], I32, kind="ExternalInput")
    msk_d = nc.dram_tensor("msk", [S, N], I32, kind="ExternalInput")
    pos_d = nc.dram_tensor("pos", [VPOS, H], F32, kind="ExternalInput")
    wn_d = nc.dram_tensor("wn", [H, H], F32, kind="ExternalInput")
    out_d = nc.dram_tensor("out", [S, H], F32, kind="ExternalOutput")

    with tile.TileContext(nc) as tc:
        with (
            tc.tile_pool(name="const", bufs=1) as constp,
            tc.tile_pool(name="stage", bufs=3) as stagep,
            tc.tile_pool(name="idxp", bufs=2) as idxp,
            tc.tile_pool(name="gbig", bufs=2) as gbigp,
            tc.tile_pool(name="outp", bufs=4) as outp,
            tc.tile_pool(name="psum", bufs=4, space="PSUM") as psump,
        ):
            # ---- Wn^T * (1/N) in bf16 --------------------------------
            wn_sb = constp.tile([H, H], F32)
            nc.sync.dma_start(wn_sb[:], wn_d[:])
            ident = constp.tile([128, 128], F32)
            make_identity(nc, ident[:])
            wnt_ps = psump.tile([128, H], F32)
            nc.tensor.transpose(out=wnt_ps[:], in_=wn_sb[:], identity=ident[:])
            wnt = constp.tile([H, H], BF16)
            nc.vector.tensor_scalar_mul(wnt[:], wnt_ps[:], 1.0 / N)

            # ---- fused table T' (bf16, gather-packed layout) ---------
            # tbl[p, q*H:(q+1)*H] = T'[q*128 + p, :]
            tbl = constp.tile([128, NRANKS * H], BF16)
            for q in range(NRANKS):
                v0 = q * 128
                n_pos = min(128, VPOS - v0)       # valid pos rows this chunk
                if n_pos <= 0:
                    nc.gpsimd.memset(tbl[:, q * H:(q + 1) * H], 0.0)
                    continue
                pstage = stagep.tile([128, H], F32, tag="pstage")
                hstage = stagep.tile([128, H], F32, tag="hstage")
                if n_pos < 128:
                    nc.gpsimd.memset(tbl[:, q * H:(q + 1) * H], 0.0)
                nc.sync.dma_start(pstage[:n_pos, :], pos_d[v0:v0 + n_pos, :])
                # h rows v0-1 .. v0+n_pos-2 ; row p needs h[v0+p-1]
                if q == 0:
                    nc.gpsimd.memset(hstage[0:1, :], 0.0)
                    nc.sync.dma_start(hstage[1:n_pos, :], h_d[0:n_pos - 1, :])
                else:
                    nc.sync.dma_start(hstage[:n_pos, :], h_d[v0 - 1:v0 + n_pos - 1, :])
                nc.vector.tensor_add(
                    tbl[:n_pos, q * H:(q + 1) * H], pstage[:n_pos, :], hstage[:n_pos, :]
                )

            # ---- wrapped masked indices (whole batch, prologue) ------
            # IMPORTANT: all 2-read DVE ops (copy_predicated) must finish
            # before any dma_gather runs -- the gather ucode streams its
            # indices through the POOL/DVE *shared* SBUF read port, and a
            # concurrent 2-port DVE op corrupts the stream. Hoisting the
            # whole index prep into the prologue makes every gather
            # transitively depend on it.
            #
            # gather call c = 8u + n_hi covers s in [128u, 128u+128) and
            # n in [4*n_hi, 4*n_hi+4); position in call i = 128*n_lo + s_lo,
            # so gbig column = 512*(n//4) + 128*(n%4) + s_lo per block.
            # Wrapped idx buffer [16, (u, n_hi, n_lo, s_hi)]:
            # idxw[p, 256u + 32*n_hi + 8*n_lo + s_hi]
            #   = idx_eff[128u + 16*s_hi + p, 4*n_hi + n_lo]
            acols = S * N // 16  # wrapped cols, whole batch
            c_sink = constp.tile([16, acols], I32)
            nc.gpsimd.memset(c_sink[:], VPOS)
            idxw32 = idxp.tile([16, acols], I32, tag="idxw32")
            mskw32 = idxp.tile([16, acols], I32, tag="mskw32")
            for u in range(S // 128):
                su = u * 128
                src_i = idx_d[su:su + 128, :].rearrange(
                    "(shi p) (nhi nlo) -> p nhi nlo shi", p=16, nlo=4)
                src_m = msk_d[su:su + 128, :].rearrange(
                    "(shi p) (nhi nlo) -> p nhi nlo shi", p=16, nlo=4)
                dst_i = idxw32[:, u * 256:(u + 1) * 256].rearrange(
                    "p (nhi nlo shi) -> p nhi nlo shi", nlo=4, shi=8)
                dst_m = mskw32[:, u * 256:(u + 1) * 256].rearrange(
                    "p (nhi nlo shi) -> p nhi nlo shi", nlo=4, shi=8)
                eng = nc.sync if u % 2 == 0 else nc.scalar
                eng.dma_start(dst_i, src_i)
                eng.dma_start(dst_m, src_m)
            idxe32 = idxp.tile([16, acols], I32, tag="idxe32")
            nc.vector.tensor_copy(idxe32[:], c_sink[:])
            nc.vector.copy_predicated(idxe32[:], mskw32[:], idxw32[:])
            # int32 -> int16 (values < 2^15: take low halves)
            idxbuf = idxp.tile([128, acols], I16, tag="idxbuf")
            lo = idxe32[:].bitcast(I16).rearrange("p (e two) -> p e two", two=2)
            nc.vector.tensor_copy(
                idxbuf[0:16, :].rearrange("p (e one) -> p e one", one=1),
                lo[:, :, 0:1],
            )
            # replicate to the 8 16-partition groups (each dma_gather queue's
            # Q7 core pair streams indices from its own 16-partition group)
            for r in range(1, 8):
                nc.sync.dma_start(idxbuf[16 * r:16 * (r + 1), :], idxbuf[0:16, :])

            for bi in range(nblk):
                s0 = bi * sblk
                wcols = sblk * N // 16  # wrapped columns per block

                # ---- gathers ----------------------------------------
                gbig = gbigp.tile([128, 1, sblk * N], BF16, tag="gbig")
                for c in range(calls):
                    wc0 = bi * wcols + c * (NI // 16)
                    nc.gpsimd.dma_gather(
                        gbig[:, :, c * NI:(c + 1) * NI],
                        tbl[:],
                        idxbuf[:, wc0:wc0 + NI // 16],
                        NI, NI, H,
                        transpose=True,
                        queue_num=c % 4,
                        sbuf_tokens_per_rank=TPR,
                        sbuf_free_dim_per_rank=H * 2,
                    )

                # ---- matmuls: psum[s,k] += g[h, s-slice]^T @ wnt -----
                gv = gbig[:, 0, :]
                for u in range(chunks):
                    ps = psump.tile([128, H], F32, tag="ps")
                    for n in range(N):
                        off = 4096 * u + 512 * (n // 4) + 128 * (n % 4)
                        nc.tensor.matmul(
                            out=ps[:],
                            lhsT=gv[:, off:off + 128],
                            rhs=wnt[:],
                            start=(n == 0),
                            stop=(n == N - 1),
                        )
                    osb = outp.tile([128, H], F32, tag="osb")
                    nc.vector.tensor_copy(osb[:], ps[:])
                    nc.sync.dma_start(
                        out_d[s0 + u * 128:s0 + (u + 1) * 128, :], osb[:]
                    )

    nc.compile()
    return nc


_CACHE: dict[int, object] = {}


def _get_program(S: int):
    if S not in _CACHE:
        _CACHE[S] = build_program(S)
    return _CACHE[S]


def kernel(x, h, g, neighbor_index, neighbor_mask, pos_table, Wn):
    """Full inputs in, full output out. x and g are unused by the math
    (g only provides the zero row shape; x is unused in the reference)."""
    h = np.asarray(h)
    idx = np.asarray(neighbor_index)
    msk = np.asarray(neighbor_mask)
    pos = np.ascontiguousarray(np.asarray(pos_table), dtype=np.float32)
    wn = np.ascontiguousarray(np.asarray(Wn), dtype=np.float32)
    b, s, n = idx.shape
    assert (b, n) == (B, N) and h.shape == (B, s, H)

    nc = _get_program(s)
    in_maps = [
        {
            "h": np.ascontiguousarray(h[c], dtype=np.float32),
            "idx": np.ascontiguousarray(idx[c], dtype=np.int32),
            "msk": np.ascontiguousarray(msk[c], dtype=np.int32),
            "pos": pos,
            "wn": wn,
        }
        for c in range(B)
    ]
    res = run_bass_kernel_spmd(nc, in_maps, core_ids=list(range(B)))
    return np.stack([res.results[c]["out"] for c in range(B)], axis=0)



# revision 3
# speedup vs baseline: 1.2611x; 1.2611x over previous
"""Trainium2 Bass kernel for nn_Neighbor_Mean (gnn message passing).

Math: out[b,s,:] = mean_n( mask[b,s,n] * (T_b[idx[b,s,n]] @ Wn^T) )
 with T_b[v] = pos_table[v] + (h[b][v-1] if v>=1 else 0)   (v in [0, 2049))
Since the mask multiplies matmul outputs and matmul is linear:
 out[b,s,:] = ( sum_n T'_b[idx_eff[b,s,n]] ) @ (Wn^T/N)
 where T' has an extra zero row at SINK=2049 and idx_eff = mask ? idx : SINK.

Sharding: data-parallel over batch, one NeuronCore per batch row (B == 8).

Per-core plan (v2 -- gather-rate bound):
 - T' in SBUF as bf16, packed [128 part, 17*256B] (row v at partition v%128,
   free chunk v//128) -- dma_gather's SBUF-source layout.
 - indices: natural wrapped layout straight from DRAM. idxw[p, 32k+n] =
   idx[16k+p, n] -- per partition 128 contiguous 128B runs (fast DMA).
   Fold mask on DVE (memset SINK + copy_predicated), narrow to int16,
   replicate to 8 16-partition groups.
 - gather call c consumes wrapped cols [32c, 32c+32) and covers
   s in [16c, 16c+16): gbig col j = 512c + 16n + p  (s = 16c + p).
   4 SWDGE queues round-robin -> ~2.4 ns/idx.
 - reduction over n on the PE via PSUM accumulation with STATIONARY
   wnt = Wn^T/N: for each 512-s block U and n: psumT[k, (cc,p)] +=
   wnt[h,k]^T @ gbig[h, 512*(32U+cc) + 16n + p]  -- psum col == s order.
 - psumT -> sbuf f32, PE-transpose 128-col chunks -> psum2[s,k] -> out rows.
"""
import sys

sys.path.insert(0, '/opt/trn_rl_repo')

import numpy as np

import concourse.bacc as bacc
import concourse.bass as bass
import concourse.mybir as mybir
import concourse.tile as tile
from concourse.bass_utils import run_bass_kernel_spmd
from concourse.masks import make_identity

B, N, H = 8, 32, 128
NI = 512             # idxs per dma_gather call (ucode ring ceiling)
UBLK = 512           # s rows per pipeline block (one PSUM accum tile)
F32 = mybir.dt.float32
I32 = mybir.dt.int32
I16 = mybir.dt.int16
BF16 = mybir.dt.bfloat16


def build_program(S: int = 2048):
    VPOS = S + 1                      # pos_table rows; SINK index == VPOS
    NRANKS = (VPOS + 1 + 127) // 128  # table chunks incl. sink row, padded
    nblk = S // UBLK if S >= UBLK else 1
    ublk = min(UBLK, S)
    calls = ublk * N // NI            # gather calls per block (s-rows/16)
    chunks = ublk // 128              # 128-s output chunks per block
    acols = S * N // 16               # wrapped idx cols, whole batch row

    nc = bacc.Bacc("TRN2", debug=False, num_swdge_queues=4)
    h_d = nc.dram_tensor("h", [S, H], F32, kind="ExternalInput")
    idx_d = nc.dram_tensor("idx", [S, N], I32, kind="ExternalInput")
    msk_d = nc.dram_tensor("msk", [S, N], I32, kind="ExternalInput")
    pos_d = nc.dram_tensor("pos", [VPOS, H], F32, kind="ExternalInput")
    wn_d = nc.dram_tensor("wn", [H, H], F32, kind="ExternalInput")
    out_d = nc.dram_tensor("out", [S, H], F32, kind="ExternalOutput")

    with tile.TileContext(nc) as tc:
        with (
            tc.tile_pool(name="const", bufs=1) as constp,
            tc.tile_pool(name="stage", bufs=3) as stagep,
            tc.tile_pool(name="prep", bufs=1) as prepp,
            tc.tile_pool(name="gbig", bufs=2) as gbigp,
            tc.tile_pool(name="outp", bufs=4) as outp,
            tc.tile_pool(name="psA", bufs=2, space="PSUM") as psA,
            tc.tile_pool(name="psB", bufs=2, space="PSUM") as psB,
        ):
            # ---- masked wrapped indices (DMA first: longest prologue) --
            # idxw[p, 32k + n] = idx[16k + p, n]; per partition 128B runs.
            idxn = prepp.tile([16, acols], I32, tag="idxn")
            mskn = prepp.tile([16, acols], I32, tag="mskn")
            nc.sync.dma_start(
                idxn[:].rearrange("p (k n) -> p k n", n=N),
                idx_d[:].rearrange("(k p) n -> p k n", p=16))
            nc.scalar.dma_start(
                mskn[:].rearrange("p (k n) -> p k n", n=N),
                msk_d[:].rearrange("(k p) n -> p k n", p=16))

            # ---- Wn^T / N in bf16 ------------------------------------
            wn_sb = constp.tile([H, H], F32)
            nc.sync.dma_start(wn_sb[:], wn_d[:])
            ident = constp.tile([128, 128], F32)
            make_identity(nc, ident[:])
            wnt_ps = psB.tile([128, H], F32, tag="wntps")
            nc.tensor.transpose(out=wnt_ps[:], in_=wn_sb[:], identity=ident[:])
            wnt = constp.tile([H, H], BF16)
            nc.vector.tensor_scalar_mul(wnt[:], wnt_ps[:], 1.0 / N)

            # ---- fused table T' (bf16, gather-packed layout) ---------
            # tbl[p, q*H:(q+1)*H] = T'[q*128 + p, :]
            tbl = constp.tile([128, NRANKS * H], BF16)
            for q in range(NRANKS):
                v0 = q * 128
                n_pos = min(128, VPOS - v0)       # valid pos rows this chunk
                if n_pos <= 0:
                    nc.gpsimd.memset(tbl[:, q * H:(q + 1) * H], 0.0)
                    continue
                pstage = stagep.tile([128, H], F32, tag="pstage")
                hstage = stagep.tile([128, H], F32, tag="hstage")
                if n_pos < 128:
                    nc.gpsimd.memset(tbl[:, q * H:(q + 1) * H], 0.0)
                nc.sync.dma_start(pstage[:n_pos, :], pos_d[v0:v0 + n_pos, :])
                # h rows v0-1 .. v0+n_pos-2 ; row p needs h[v0+p-1]
                if q == 0:
                    nc.gpsimd.memset(hstage[0:1, :], 0.0)
                    nc.sync.dma_start(hstage[1:n_pos, :], h_d[0:n_pos - 1, :])
                else:
                    nc.scalar.dma_start(
                        hstage[:n_pos, :], h_d[v0 - 1:v0 + n_pos - 1, :])
                nc.vector.tensor_add(
                    tbl[:n_pos, q * H:(q + 1) * H], pstage[:n_pos, :], hstage[:n_pos, :]
                )

            # ---- mask fold + int16 narrowing on DVE ------------------
            # IMPORTANT: copy_predicated (2-read DVE) must finish before
            # any dma_gather runs (shared POOL/DVE SBUF read port). All
            # gathers depend on idxbuf, which depends on these ops.
            idxe = prepp.tile([16, acols], I32, tag="idxe")
            nc.vector.memset(idxe[:], VPOS)
            nc.vector.copy_predicated(idxe[:], mskn[:], idxn[:])
            idxbuf = prepp.tile([128, acols], I16, tag="idxbuf")
            lo = idxe[:].bitcast(I16).rearrange("p (e two) -> p e two", two=2)
            nc.vector.tensor_copy(
                idxbuf[0:16, :].rearrange("p (e one) -> p e one", one=1),
                lo[:, :, 0:1],
            )
            # replicate to the 8 16-partition groups
            for r in range(1, 8):
                eng = nc.sync if r % 2 == 0 else nc.scalar
                eng.dma_start(idxbuf[16 * r:16 * (r + 1), :], idxbuf[0:16, :])

            for bi in range(nblk):
                s0 = bi * ublk
                # ---- gathers ----------------------------------------
                gbig = gbigp.tile([128, 1, ublk * N], BF16, tag="gbig")
                for c in range(calls):
                    wc0 = (s0 // 16 + c) * (NI // 16)
                    nc.gpsimd.dma_gather(
                        gbig[:, :, c * NI:(c + 1) * NI],
                        tbl[:],
                        idxbuf[:, wc0:wc0 + NI // 16],
                        NI, NI, H,
                        transpose=True,
                        queue_num=c % 4,
                        sbuf_tokens_per_rank=128,
                        sbuf_free_dim_per_rank=H * 2,
                    )

                # ---- n-sum on PE: psumT[k, (cc,p)] += wnt^T @ g ------
                # gv view [h, cc, n, p]: col j = 512*cc + 16*n + p
                gv = gbig[:, 0, :].rearrange(
                    "h (cc n p) -> h cc n p", n=N, p=16)
                psumT = psA.tile([128, ublk], F32, tag="psumT")
                for n in range(N):
                    nc.tensor.matmul(
                        out=psumT[:],
                        lhsT=wnt[:],
                        rhs=gv[:, :, n, :],
                        start=(n == 0),
                        stop=(n == N - 1),
                    )
                # ---- transpose to [s, k] and write out ---------------
                sbufT = outp.tile([128, ublk], F32, tag="sbufT")
                nc.vector.tensor_copy(sbufT[:], psumT[:])
                for i in range(chunks):
                    ps2 = psB.tile([128, 128], F32, tag="ps2")
                    nc.tensor.transpose(
                        out=ps2[:], in_=sbufT[:, i * 128:(i + 1) * 128],
                        identity=ident[:])
                    osb = outp.tile([128, H], F32, tag="osb")
                    nc.vector.tensor_copy(osb[:], ps2[:])
                    eng = nc.sync if i % 2 == 0 else nc.scalar
                    eng.dma_start(
                        out_d[s0 + i * 128:s0 + (i + 1) * 128, :], osb[:]
                    )

    nc.compile()
    return nc


_CACHE: dict[int, object] = {}


def _get_program(S: int):
    if S not in _CACHE:
        _CACHE[S] = build_program(S)
    return _CACHE[S]


def kernel(x, h, g, neighbor_index, neighbor_mask, pos_table, Wn):
    """Full inputs in, full output out. x and g are unused by the math
    (g only provides the zero row shape; x is unused in the reference)."""
    h = np.asarray(h)
    idx = np.asarray(neighbor_index)
    msk = np.asarray(neighbor_mask)
    pos = np.ascontiguousarray(np.asarray(pos_table), dtype=np.float32)
    wn = np.ascontiguousarray(np.asarray(Wn), dtype=np.float32)
    b, s, n = idx.shape
    assert (b, n) == (B, N) and h.shape == (B, s, H)

    nc = _get_program(s)
    in_maps = [
        {
            "h": np.ascontiguousarray(h[c], dtype=np.float32),
            "idx": np.ascontiguousarray(idx[c], dtype=np.int32),
            "msk": np.ascontiguousarray(msk[c], dtype=np.int32),
            "pos": pos,
            "wn": wn,
        }
        for c in range(B)
    ]
    res = run_bass_kernel_spmd(nc, in_maps, core_ids=list(range(B)))
    return np.stack([res.results[c]["out"] for c in range(B)], axis=0)


# revision 14
# speedup vs baseline: 2.3936x; 1.8980x over previous
"""Trainium2 Bass kernel for nn_Neighbor_Mean (gnn message passing).

Math: out[b,s,:] = mean_n( mask[b,s,n] * (T_b[idx[b,s,n]] @ Wn^T) )
 with T_b[v] = pos_table[v] + (h[b][v-1] if v>=1 else 0)   (v in [0, 2049))
Since the mask multiplies matmul outputs and matmul is linear:
 out[b,s,:] = ( sum_n T'_b[idx_eff[b,s,n]] ) @ (Wn^T/N)
 where T' has zero rows at 2049..2175 and idx_eff = mask ? idx : sink,
 with the sink SPREAD over the zero rows (2049+2*(col%64)) so masked
 gathers don't serialize on one HBM/SBUF row.

Sharding: data-parallel over batch, one NeuronCore per batch row (B == 8).

Per-core plan (v3):
 - T' built in SBUF bf16 then copied to a DRAM scratch table [2176, 128].
 - dma_gather HBM-source NON-transpose (4 SWDGE queues, ~2.4 ns/idx).
   The transposed (XBAR) gather corrupts data non-deterministically at
   full rate (verified on HW); the plain row-per-partition path is clean.
 - gather call c covers s-chunk U=c//8, n in [4*(c%8), +4); position
   i = 128*a + k -> gblk[k, block, :] = T'[idx_eff[128U + k, 4*(c%8)+a]]
   i.e. each 128-row block is one (U, n) pair with s along partitions.
 - n-sum on PE: psum[s,h] += I^T @ block  (32 identity-matmuls/chunk).
 - per chunk: m -> bf16 -> PE transpose -> mT; out[s,k] = mT^T @ wnt.
 - All mid-stream copies on the Activation engine: concurrent DVE ops
   can corrupt the gather ucode's index stream (shared POOL/DVE port).
"""
import sys

sys.path.insert(0, '/opt/trn_rl_repo')

import numpy as np

import concourse.bacc as bacc
import concourse.bass as bass
import concourse.mybir as mybir
import concourse.tile as tile
from concourse.bass_utils import run_bass_kernel_spmd
from concourse.masks import make_identity

B, N, H = 8, 32, 128
NI = 512             # idxs per dma_gather call (ucode ring ceiling)
UBLK = 512           # s rows per pipeline block
F32 = mybir.dt.float32
I32 = mybir.dt.int32
I16 = mybir.dt.int16
BF16 = mybir.dt.bfloat16


def build_program(S: int = 2048, mode: str = "full"):
    VPOS = S + 1                      # pos_table rows; sinks start at VPOS
    NRANKS = (VPOS + 1 + 127) // 128  # table 128-row chunks, zero padded
    VPAD = NRANKS * 128
    nblk = S // UBLK if S >= UBLK else 1
    ublk = min(UBLK, S)
    calls = ublk * N // NI            # gather calls per pipeline block
    chunks = ublk // 128              # 128-s chunks per pipeline block
    acols = S * N // 16               # wrapped idx cols, whole batch row

    nc = bacc.Bacc("TRN2", debug=False, num_swdge_queues=4)
    h_d = nc.dram_tensor("h", [S, H], F32, kind="ExternalInput")
    idx_d = nc.dram_tensor("idx", [S, N], I32, kind="ExternalInput")
    msk_d = nc.dram_tensor("msk", [S, N], I32, kind="ExternalInput")
    pos_d = nc.dram_tensor("pos", [VPOS, H], F32, kind="ExternalInput")
    wn_d = nc.dram_tensor("wn", [H, H], F32, kind="ExternalInput")
    out_d = nc.dram_tensor("out", [S, H], F32, kind="ExternalOutput")
    tbl_d = nc.dram_tensor("tscratch", [VPAD, H], BF16, kind="Internal")
    dump_d = None
    if mode == "nomm":
        dump_d = nc.dram_tensor("gdump", [128, S * N], mybir.dt.uint16,
                                kind="ExternalOutput")

    with tile.TileContext(nc) as tc:
        with (
            tc.tile_pool(name="const", bufs=1) as constp,
            tc.tile_pool(name="stage", bufs=3) as stagep,
            tc.tile_pool(name="prep", bufs=1) as prepp,
            tc.tile_pool(name="gblk", bufs=2) as gblkp,
            tc.tile_pool(name="outp", bufs=4) as outp,
            tc.tile_pool(name="psA", bufs=2, space="PSUM") as psA,
            tc.tile_pool(name="psB", bufs=2, space="PSUM") as psB,
            tc.tile_pool(name="psC", bufs=2, space="PSUM") as psC,
        ):
            # ---- raw wrapped indices: longest DMA first ---------------
            # natural layout: idxn[p, 32k + n] = idx[16k + p, n]
            # (per partition 128 contiguous 128B runs). Replicate the RAW
            # i32 data to all 8 groups, then all DVE prep runs on 128
            # partitions at once and writes idxbuf in gather order.
            idxn = prepp.tile([128, acols], I32, tag="idxn")
            mskn = prepp.tile([128, acols], I32, tag="mskn")
            nc.sync.dma_start(
                idxn[0:16, :].rearrange("p (k n) -> p k n", n=N),
                idx_d[:].rearrange("(k p) n -> p k n", p=16))
            nc.scalar.dma_start(
                mskn[0:16, :].rearrange("p (k n) -> p k n", n=N),
                msk_d[:].rearrange("(k p) n -> p k n", p=16))
            for r in range(1, 8):
                eng = nc.sync if r % 2 == 0 else nc.scalar
                eng.dma_start(idxn[16 * r:16 * (r + 1), :], idxn[0:16, :])
                eng2 = nc.scalar if r % 2 == 0 else nc.sync
                eng2.dma_start(mskn[16 * r:16 * (r + 1), :], mskn[0:16, :])

            # ---- Wn^T / N in bf16 ------------------------------------
            wn_sb = constp.tile([H, H], F32)
            nc.sync.dma_start(wn_sb[:], wn_d[:])
            ident = constp.tile([128, 128], F32)
            make_identity(nc, ident[:])
            identb = constp.tile([128, 128], BF16)
            nc.vector.tensor_copy(identb[:], ident[:])
            wnt_ps = psC.tile([128, H], F32, tag="wntps")
            nc.tensor.transpose(out=wnt_ps[:], in_=wn_sb[:], identity=ident[:])
            wnt = constp.tile([H, H], BF16)
            nc.vector.tensor_scalar_mul(wnt[:], wnt_ps[:], 1.0 / N)

            # ---- fused table T' -> SBUF bf16 -> DRAM scratch ---------
            # tbl[p, q*H:(q+1)*H] = T'[q*128 + p, :]; rows VPOS..VPAD-1 = 0
            tbl = constp.tile([128, NRANKS * H], BF16)
            for q in range(NRANKS):
                v0 = q * 128
                n_pos = min(128, VPOS - v0)
                if n_pos <= 0:
                    nc.gpsimd.memset(tbl[:, q * H:(q + 1) * H], 0.0)
                    continue
                pstage = stagep.tile([128, H], F32, tag="pstage")
                hstage = stagep.tile([128, H], F32, tag="hstage")
                if n_pos < 128:
                    nc.gpsimd.memset(tbl[:, q * H:(q + 1) * H], 0.0)
                nc.sync.dma_start(pstage[:n_pos, :], pos_d[v0:v0 + n_pos, :])
                if q == 0:
                    nc.gpsimd.memset(hstage[0:1, :], 0.0)
                    nc.sync.dma_start(hstage[1:n_pos, :], h_d[0:n_pos - 1, :])
                else:
                    nc.scalar.dma_start(
                        hstage[:n_pos, :], h_d[v0 - 1:v0 + n_pos - 1, :])
                nc.vector.tensor_add(
                    tbl[:n_pos, q * H:(q + 1) * H], pstage[:n_pos, :], hstage[:n_pos, :]
                )
            nc.sync.dma_start(
                tbl_d[:].rearrange("(q p) e -> p q e", p=128),
                tbl[:].rearrange("p (q e) -> p q e", e=H))

            # ---- mask fold + narrow + permute (prologue DVE) ---------
            # masked -> spread sink rows 2049+2*(col%64) (zero table rows)
            idxe = prepp.tile([128, acols], I32, tag="idxe")
            nc.gpsimd.iota(
                idxe[:].rearrange("p (r c) -> p r c", c=64),
                pattern=[[0, acols // 64], [2, 64]],
                base=VPOS,
                channel_multiplier=0,
            )
            nc.vector.copy_predicated(idxe[:], mskn[:], idxn[:])
            idxn16 = prepp.tile([128, acols], I16, tag="idxn16")
            lo = idxe[:].bitcast(I16).rearrange("p (e two) -> p e two", two=2)
            nc.vector.tensor_copy(
                idxn16[:].rearrange("p (e one) -> p e one", one=1),
                lo[:, :, 0:1],
            )
            # permute natural (kk nh a) -> gather order (nh a kk) per U:
            # idxbuf[p, 256U + 32nh + 8a + kk] = idx_eff[128U + 16kk + p,
            #                                            4nh + a]
            idxbuf = prepp.tile([128, acols], I16, tag="idxbuf")
            for u in range(S // 128):
                sl = slice(256 * u, 256 * (u + 1))
                nc.vector.tensor_copy(
                    idxbuf[:, sl].rearrange(
                        "p (nh a kk) -> p nh a kk", a=4, kk=8),
                    idxn16[:, sl].rearrange(
                        "p (kk nh a) -> p nh a kk", nh=8, a=4),
                )

            for bi in range(nblk):
                s0 = bi * ublk
                # ---- gathers: call c -> 4 blocks of (U, n) rows ------
                gblk = gblkp.tile([128, chunks * N, H], BF16, tag="gblk")
                for c in range(calls):
                    cg = bi * calls + c         # global call number
                    wc0 = cg * (NI // 16)
                    nc.gpsimd.dma_gather(
                        gblk[:, 4 * c:4 * c + 4, :],
                        tbl_d[:],
                        idxbuf[:, wc0:wc0 + NI // 16],
                        NI, NI, H,
                        transpose=False,
                        queue_num=cg % 4,
                    )

                if mode == "nomm":
                    nc.scalar.dma_start(
                        dump_d[:].rearrange(
                            "p (x e) -> p x e", e=H)[:, s0 * N // 128:
                                                     (s0 + ublk) * N // 128, :],
                        gblk[:].bitcast(mybir.dt.uint16))
                    continue

                # ---- per 128-s chunk: n-sum, transpose, Wn -----------
                for uu in range(chunks):
                    psm = psA.tile([128, H], F32, tag="psm")
                    for n in range(N):
                        nc.tensor.matmul(
                            out=psm[:],
                            lhsT=identb[:],
                            rhs=gblk[:, N * uu + n, :],
                            start=(n == 0),
                            stop=(n == N - 1),
                        )
                    msb = outp.tile([128, H], BF16, tag="msb")
                    nc.scalar.copy(msb[:], psm[:])
                    pst = psB.tile([128, H], BF16, tag="pst")
                    nc.tensor.transpose(
                        out=pst[:], in_=msb[:], identity=identb[:])
                    mT = outp.tile([128, H], BF16, tag="mT")
                    nc.scalar.copy(mT[:], pst[:])
                    pso = psC.tile([128, H], F32, tag="pso")
                    nc.tensor.matmul(
                        out=pso[:], lhsT=mT[:], rhs=wnt[:],
                        start=True, stop=True)
                    osb = outp.tile([128, H], F32, tag="osb")
                    nc.scalar.copy(osb[:], pso[:])
                    eng = nc.sync if uu % 2 == 0 else nc.scalar
                    eng.dma_start(
                        out_d[s0 + uu * 128:s0 + (uu + 1) * 128, :], osb[:]
                    )

    nc.compile()
    return nc


_CACHE: dict[int, object] = {}


def _get_program(S: int):
    if S not in _CACHE:
        _CACHE[S] = build_program(S)
    return _CACHE[S]


def kernel(x, h, g, neighbor_index, neighbor_mask, pos_table, Wn):
    """Full inputs in, full output out. x and g are unused by the math
    (g only provides the zero row shape; x is unused in the reference)."""
    h = np.asarray(h)
    idx = np.asarray(neighbor_index)
    msk = np.asarray(neighbor_mask)
    pos = np.ascontiguousarray(np.asarray(pos_table), dtype=np.float32)
    wn = np.ascontiguousarray(np.asarray(Wn), dtype=np.float32)
    b, s, n = idx.shape
    assert (b, n) == (B, N) and h.shape == (B, s, H)

    nc = _get_program(s)
    in_maps = [
        {
            "h": np.ascontiguousarray(h[c], dtype=np.float32),
            "idx": np.ascontiguousarray(idx[c], dtype=np.int32),
            "msk": np.ascontiguousarray(msk[c], dtype=np.int32),
            "pos": pos,
            "wn": wn,
        }
        for c in range(B)
    ]
    res = run_bass_kernel_spmd(nc, in_maps, core_ids=list(range(B)))
    return np.stack([res.results[c]["out"] for c in range(B)], axis=0)
